# revision 25
# baseline (speedup 1.0000x reference)
"""Trainium2 Bass kernel for the ConvModule problem (DFT8 conv version).

Computes, for x (B=16, T=1024, C=512) fp32:
    h = LayerNorm_C(x) -> pw conv C->2C + Swish -> k=5 conv 2C->2C
      -> GLU -> BatchNorm(eval) -> pw conv C->C
Data-parallel over batch across 8 NeuronCores (2 batches/core, weights
replicated).  LN gamma/beta folded into w1/b1, BN folded into w3/b3 on the
host.

The k=5 'same' conv is computed as a length-8 cyclic correlation per tile of
4 outputs (exact since 3+4 <= 7), via a real FFT8 on the device (DVE/GpSimd
butterflies on stride-1 deinterleaved planes) and host-side transformed
weights U = conj(FFT8(w2 zero-padded))/8.  Per complex point j, three U
planes (Re, -Im, +Im) are stored so every PSUM contribution is a plain
accumulate:
    M_jr = A_j V_jr + B_j V_ji ,  M_ji = C_j V_jr + A_j V_ji
with A=Re(U), B=-Im(U), C=+Im(U).  This needs 14 GEMM passes per 4 outputs
instead of the direct method's 20, cutting Tensor-engine time ~1.4x.
"""

from contextlib import ExitStack

import numpy as np

import concourse.bass as bass
import concourse.bacc as bacc
import concourse.tile as tile
from concourse import mybir
from concourse.masks import make_identity
from concourse.bass_utils import run_bass_kernel_spmd

B, T, C, K = 16, 1024, 512, 5
EPS_LN = 1e-5
EPS_BN = 1e-5
NCORES = 8
BLOC = B // NCORES          # batches per core
P = 128                     # SBUF partitions
CB = C // P                 # 4 channel blocks of the C dim
OB = (2 * C) // P           # 8 channel blocks of the 2C dim
TH = T // 2                 # 512
NT = T // 4                 # 256 conv tiles per batch (4 outputs each)
NU = 11                     # stored U planes: U0, U4, (A,B,C) x j=1..3
F32 = mybir.dt.float32
BF16 = mybir.dt.bfloat16
RS2 = float(1.0 / np.sqrt(2.0))
SQ2 = float(np.sqrt(2.0))

AF = mybir.ActivationFunctionType
ALU = mybir.AluOpType

# GEMM pass lists: (psum plane index, [(u_idx, v_name), ...])
# u planes: 0:U0 1:U4 2:A1 3:B1 4:C1 5:A2 6:B2 7:C2 8:A3 9:B3 10:C3
# E group: M0, M4, M2r, M2i ; O group: M1r, M1i, M3r, M3i
MPASS_E = [
    (0, [(0, "v0")]),
    (1, [(1, "v4")]),
    (2, [(5, "v2r"), (6, "v2i")]),
    (3, [(7, "v2r"), (5, "v2i")]),
]
MPASS_O = [
    (0, [(2, "v1r"), (3, "v1i")]),
    (1, [(4, "v1r"), (2, "v1i")]),
    (2, [(8, "v3r"), (9, "v3i")]),
    (3, [(10, "v3r"), (8, "v3i")]),
]
VNAMES = ["v0", "v4", "v1r", "v1i", "v2r", "v2i", "v3r", "v3i"]


def build_nc() -> bass.Bass:
    nc = bacc.Bacc("TRN2")

    xs = nc.declare_dram_parameter("xs", [BLOC, T, C], BF16, isOutput=False)
    w1t = nc.declare_dram_parameter("w1t", [CB, P, 2 * C], BF16, isOutput=False)
    ut = nc.declare_dram_parameter("ut", [OB, P, NU, OB, P], BF16, isOutput=False)
    w3t = nc.declare_dram_parameter("w3t", [CB, P, C], BF16, isOutput=False)
    b1 = nc.declare_dram_parameter("b1", [P, OB], F32, isOutput=False)
    b2 = nc.declare_dram_parameter("b2", [P, OB], F32, isOutput=False)
    b3 = nc.declare_dram_parameter("b3", [P, C], F32, isOutput=False)
    out = nc.declare_dram_parameter("out", [BLOC, T, C], F32, isOutput=True)

    with ExitStack() as ctx:
        tc = ctx.enter_context(tile.TileContext(nc))

        consts = ctx.enter_context(tc.tile_pool(name="consts", bufs=1))
        xin = ctx.enter_context(tc.tile_pool(name="xin", bufs=2))
        stats = ctx.enter_context(tc.tile_pool(name="stats", bufs=4))
        hNp = ctx.enter_context(tc.tile_pool(name="hNp", bufs=1))
        h1rp = ctx.enter_context(tc.tile_pool(name="h1rp", bufs=3))
        fsc = ctx.enter_context(tc.tile_pool(name="fsc", bufs=1))
        vpool = ctx.enter_context(tc.tile_pool(name="vpool", bufs=1))
        upool = ctx.enter_context(tc.tile_pool(name="upool", bufs=2))
        mcp = ctx.enter_context(tc.tile_pool(name="mcp", bufs=2))
        isc = ctx.enter_context(tc.tile_pool(name="isc", bufs=1))
        yap = ctx.enter_context(tc.tile_pool(name="yap", bufs=2))
        hGp = ctx.enter_context(tc.tile_pool(name="hGp", bufs=1))
        outp = ctx.enter_context(tc.tile_pool(name="outp", bufs=2))
        cv_psum = ctx.enter_context(tc.tile_pool(name="cv_psum", bufs=2, space="PSUM"))
        ab_psum = ctx.enter_context(tc.tile_pool(name="ab_psum", bufs=2, space="PSUM"))
        o_psum = ctx.enter_context(tc.tile_pool(name="o_psum", bufs=2, space="PSUM"))

        # ---- constants / persistent weights ----
        ident = consts.tile([P, P], BF16, tag="ident")
        make_identity(nc, ident)
        epssb = consts.tile([P, 1], F32, tag="eps")
        nc.vector.memset(epssb, EPS_LN)
        b1sb = consts.tile([P, OB], F32, tag="b1")
        nc.sync.dma_start(out=b1sb, in_=b1[:])
        b2sb = consts.tile([P, OB], F32, tag="b2")
        nc.sync.dma_start(out=b2sb, in_=b2[:])
        b3sb = consts.tile([P, C], F32, tag="b3")
        nc.sync.dma_start(out=b3sb, in_=b3[:])
        w1sb = []
        for cb in range(CB):
            w = consts.tile([P, 2 * C], BF16, tag=f"w1_{cb}", name=f"w1_{cb}")
            nc.sync.dma_start(out=w, in_=w1t[cb])
            w1sb.append(w)
        w3sb = []
        for cb in range(CB):
            w = consts.tile([P, C], BF16, tag=f"w3_{cb}", name=f"w3_{cb}")
            nc.sync.dma_start(out=w, in_=w3t[cb])
            w3sb.append(w)

        # V planes: [128, BLOC, NT] bf16 per (plane, ib); halves written per batch
        vsb = {}
        for vn in VNAMES:
            for ib in range(OB):
                v = vpool.tile([P, BLOC, NT], BF16, tag=f"{vn}_{ib}",
                               name=f"{vn}_{ib}")
                vsb[(vn, ib)] = v

        hg_tiles = {}
        hN_all = {}

        # ---------- Phase A: LN + transpose to [c, t] ----------
        def phase_A(b):
            xh = []
            xsr = xs[b].rearrange("(h tb p) c -> h p tb c", h=4, p=P)
            for h in range(4):
                xb = xin.tile([P, T // P // 4, C], BF16, tag="xbig",
                              name=f"xbig_{b}_{h}")
                nc.gpsimd.dma_start(out=xb, in_=xsr[h])
                xh.append(xb)
            hN = hNp.tile([P, CB * T], BF16, tag="hN", name=f"hN_{b}")
            hN_all[b] = hN
            hN3 = hN[:, :].rearrange("p (c t) -> p c t", c=CB)
            for tb in range(T // P):
                xt = xh[tb // 2][:, tb % 2, :]
                st6 = stats.tile([P, 6], F32, tag="st6")
                nc.vector.bn_stats(out=st6, in_=xt)
                mv = stats.tile([P, 2], F32, tag="mv")
                nc.vector.bn_aggr(out=mv, in_=st6)
                rstd = stats.tile([P, 1], F32, tag="rstd")
                nc.scalar.activation(
                    out=rstd, in_=mv[:, 1:2], func=AF.Sqrt,
                    bias=epssb, scale=1.0,
                )
                nc.vector.reciprocal(out=rstd, in_=rstd)
                nmu = stats.tile([P, 1], F32, tag="nmu")
                nc.vector.tensor_scalar(
                    out=nmu, in0=mv[:, 0:1], scalar1=rstd, scalar2=-1.0,
                    op0=ALU.mult, op1=ALU.mult,
                )
                xn = stats.tile([P, C], BF16, tag="xn")
                nc.scalar.activation(
                    out=xn, in_=xt, func=AF.Identity, bias=nmu, scale=rstd,
                )
                ps = ab_psum.tile([P, TH], BF16, tag="ab", name=f"tp_{b}_{tb}")
                for cb in range(CB):
                    nc.tensor.transpose(
                        ps[:, cb * P:(cb + 1) * P], xn[:, cb * P:(cb + 1) * P],
                        ident,
                    )
                nc.scalar.copy(
                    out=hN3[:, :, tb * P:(tb + 1) * P],
                    in_=ps[:, 0:CB * P].rearrange("p (c i) -> p c i", c=CB),
                )

        # ---------- Phase B: conv1 C->2C + Swish into deinterleaved planes ----
        def phase_B(b):
            hN3 = hN_all[b][:, :].rearrange("p (c t) -> p c t", c=CB)
            h1r = []
            for ob in range(OB):
                # q_i[u] = h1[4u + i - 2]: 8 aligned planes (duplicated halo)
                t_ = h1rp.tile([P, 8, NT], BF16, tag="h1r",
                               name=f"h1r_{ob}_{b}")
                nc.vector.memset(t_[:, 0:2, 0:1], 0.0)
                nc.vector.memset(t_[:, 6:8, NT - 1:NT], 0.0)
                # Silu(z + b1) -> planes 2..5: t = 4u+j lands at q_{j+2}[u]
                for ph in range(2):
                    pz = ab_psum.tile([P, TH], F32, tag="ab",
                                      name=f"pz_{ob}_{b}_{ph}")
                    for cb in range(CB):
                        w = w1sb[cb][:, ob * P:(ob + 1) * P]
                        nc.tensor.matmul(
                            pz, w, hN3[:, cb, ph * TH:(ph + 1) * TH],
                            start=(cb == 0), stop=(cb == CB - 1))
                    dst = t_[:, 2:6, 128 * ph:128 * ph + 128]
                    nc.scalar.activation(
                        out=dst.rearrange("p j u -> p u j"),
                        in_=pz,
                        func=AF.Silu, bias=b1sb[:, ob:ob + 1], scale=1.0,
                    )
                # duplicated shifted planes (gpsimd copies)
                nc.gpsimd.tensor_copy(t_[:, 0, 1:NT], t_[:, 4, 0:NT - 1])
                nc.gpsimd.tensor_copy(t_[:, 1, 1:NT], t_[:, 5, 0:NT - 1])
                nc.gpsimd.tensor_copy(t_[:, 6, 0:NT - 1], t_[:, 2, 1:NT])
                nc.gpsimd.tensor_copy(t_[:, 7, 0:NT - 1], t_[:, 3, 1:NT])
                h1r.append(t_)
            return h1r

        # ---------- FFT8 per (ib, batch): h1r -> 8 V planes ----------
        def fft_ib(r, ib, b):
            d = [r[:, i, 0:NT] for i in range(8)]
            s, t_ = [], []
            for i in range(4):
                si = fsc.tile([P, NT], BF16, tag=f"s{i}", name=f"s{i}_{ib}_{b}")
                nc.gpsimd.tensor_add(out=si, in0=d[i], in1=d[i + 4])
                s.append(si)
                ti = fsc.tile([P, NT], BF16, tag=f"t{i}", name=f"t{i}_{ib}_{b}")
                nc.vector.tensor_sub(out=ti, in0=d[i], in1=d[i + 4])
                t_.append(ti)
            u0 = fsc.tile([P, NT], BF16, tag="u0", name=f"u0_{ib}_{b}")
            nc.gpsimd.tensor_add(out=u0, in0=s[0], in1=s[2])
            u1 = fsc.tile([P, NT], BF16, tag="u1", name=f"u1_{ib}_{b}")
            nc.gpsimd.tensor_add(out=u1, in0=s[1], in1=s[3])
            V = {vn: vsb[(vn, ib)][:, b, :] for vn in VNAMES}
            nc.gpsimd.tensor_add(out=V["v0"], in0=u0, in1=u1)
            nc.gpsimd.tensor_sub(out=V["v4"], in0=u0, in1=u1)
            nc.gpsimd.tensor_sub(out=V["v2r"], in0=s[0], in1=s[2])
            nc.gpsimd.tensor_sub(out=V["v2i"], in0=s[3], in1=s[1])
            a = fsc.tile([P, NT], BF16, tag="fa", name=f"fa_{ib}_{b}")
            nc.vector.tensor_sub(out=a, in0=t_[1], in1=t_[3])
            bb = fsc.tile([P, NT], BF16, tag="fb", name=f"fb_{ib}_{b}")
            nc.vector.tensor_add(out=bb, in0=t_[1], in1=t_[3])
            nc.vector.scalar_tensor_tensor(
                out=V["v1r"], in0=a, scalar=RS2, in1=t_[0],
                op0=ALU.mult, op1=ALU.add)
            nc.vector.scalar_tensor_tensor(
                out=V["v3r"], in0=a, scalar=-RS2, in1=t_[0],
                op0=ALU.mult, op1=ALU.add)
            nc.vector.scalar_tensor_tensor(
                out=V["v1i"], in0=bb, scalar=-RS2, in1=t_[2],
                op0=ALU.mult, op1=ALU.subtract)
            nc.vector.scalar_tensor_tensor(
                out=V["v3i"], in0=bb, scalar=-RS2, in1=t_[2],
                op0=ALU.mult, op1=ALU.add)

        # ---------- conv GEMM + IFFT per ob (both batches share LDW) ----------
        def gemm_group(mms, passes, usb):
            for pl, plist in passes:
                n = len(plist) * OB
                i = 0
                for (ui, vn) in plist:
                    for ib in range(OB):
                        for b in range(BLOC):
                            nc.tensor.matmul(
                                mms[b][:, pl, :],
                                usb[:, ui, ib, :],
                                vsb[(vn, ib)][:, b, :],
                                start=(i == 0), stop=(i == n - 1),
                            )
                        i += 1

        def conv_ob(ob, usb):
            def stt(out_, in0, sc, in1):
                nc.vector.scalar_tensor_tensor(
                    out=out_, in0=in0, scalar=sc, in1=in1,
                    op0=ALU.mult, op1=ALU.add)

            def tl(tag, b, bufs=None):
                return isc.tile([P, NT], BF16, tag=tag, name=f"{tag}_{ob}_{b}",
                                bufs=bufs)

            mcs = [mcp.tile([P, 8, NT], BF16, tag="mc", name=f"mc_{ob}_{b}")
                   for b in range(BLOC)]
            # E group: M0, M4, M2r, M2i
            mmE = [cv_psum.tile([P, 4, NT], F32, tag="mm", name=f"mmE_{ob}_{b}")
                   for b in range(BLOC)]
            gemm_group(mmE, MPASS_E, usb)
            Es = {}
            for b in range(BLOC):
                nc.scalar.copy(out=mcs[b][:, 0:4, :], in_=mmE[b][:, :, :])
                mc = mcs[b]
                M0, M4 = mc[:, 0, :], mc[:, 1, :]
                M2r, M2i = mc[:, 2, :], mc[:, 3, :]
                Pt, Qt = tl("iP", b), tl("iQ", b)
                stt(Pt, M4, 1.0, M0)
                stt(Qt, M4, -1.0, M0)
                E = []
                for k, (msrc, sc, base) in enumerate(
                        ((M2r, 2.0, Pt), (M2i, -2.0, Qt),
                         (M2r, -2.0, Pt), (M2i, 2.0, Qt))):
                    e = tl(f"iE{k}", b, bufs=2)
                    stt(e, msrc, sc, base)
                    E.append(e)
                Es[b] = E
            # O group: M1r, M1i, M3r, M3i
            mmO = [cv_psum.tile([P, 4, NT], F32, tag="mm", name=f"mmO_{ob}_{b}")
                   for b in range(BLOC)]
            gemm_group(mmO, MPASS_O, usb)
            ys = {}
            for b in range(BLOC):
                nc.scalar.copy(out=mcs[b][:, 4:8, :], in_=mmO[b][:, :, :])
                mc = mcs[b]
                M1r, M1i = mc[:, 4, :], mc[:, 5, :]
                M3r, M3i = mc[:, 6, :], mc[:, 7, :]
                E = Es[b]
                y = yap.tile([P, 4, NT], BF16, tag="ya" if ob < CB else "yg",
                             name=f"y_{ob}_{b}")
                w0 = tl("iw0", b)
                stt(w0, M3r, 1.0, M1r)
                stt(y[:, 0, :], w0, 2.0, E[0])
                aa, bb2, e1 = tl("iaa", b), tl("ibb", b), tl("ie1", b)
                stt(aa, M1i, -1.0, M1r)          # M1r - M1i
                stt(bb2, M3i, 1.0, M3r)          # M3r + M3i
                stt(e1, bb2, -1.0, aa)           # aa - bb2
                stt(y[:, 1, :], e1, SQ2, E[1])
                w2_ = tl("iw2", b)
                stt(w2_, M1i, -1.0, M3i)         # M3i - M1i
                stt(y[:, 2, :], w2_, 2.0, E[2])
                cc, dd, e3 = tl("icc", b), tl("idd", b), tl("ie3", b)
                stt(cc, M1i, 1.0, M1r)           # M1r + M1i
                stt(dd, M3i, -1.0, M3r)          # M3r - M3i
                stt(e3, dd, -1.0, cc)            # cc - dd
                stt(y[:, 3, :], e3, -SQ2, E[3])
                ys[b] = y
            return ys

        # ---------- GLU per (value-ob v, batch) ----------
        def glu(v, b, ya, yg):
            sg = isc.tile([P, 4, NT], BF16, tag="sg", name=f"sg_{v}_{b}", bufs=2)
            nc.scalar.activation(
                out=sg.rearrange("p j u -> p (j u)"),
                in_=yg.rearrange("p j u -> p (j u)"),
                func=AF.Sigmoid, bias=b2sb[:, v + CB:v + CB + 1], scale=1.0,
            )
            hg = hGp.tile([P, T], BF16, tag=f"hg{v}", name=f"hg{v}_{b}", bufs=2)
            hg_tiles[(v, b)] = hg
            # hg[4u+j] = (ya[j,u] + b2[v]) * sg[j,u]  (scatter to t-seq layout)
            nc.vector.scalar_tensor_tensor(
                out=hg.rearrange("p (u j) -> p j u", j=4),
                in0=ya, scalar=b2sb[:, v:v + 1], in1=sg,
                op0=ALU.add, op1=ALU.mult,
            )

        # ---------- Phase D: conv3 with activations stationary ----------
        def phase_D(b):
            for tb in range(T // P):
                po = o_psum.tile([P, C], F32, tag="po", name=f"po_{b}_{tb}")
                for cb in range(CB):
                    hg = hg_tiles[(cb, b)]
                    nc.tensor.matmul(
                        po, hg[:, P * tb:P * (tb + 1)], w3sb[cb],
                        start=(cb == 0), stop=(cb == CB - 1),
                    )
                obig = outp.tile([P, C], F32, tag="obig", name=f"ob_{b}_{tb}")
                nc.vector.tensor_add(out=obig, in0=po, in1=b3sb)
                nc.gpsimd.dma_start(
                    out=out[b].rearrange("(tb p) c -> p tb c", p=P)[:, tb, :],
                    in_=obig,
                )

        # ================= schedule =================
        for b in range(BLOC):
            phase_A(b)
            h1r = phase_B(b)
            for ib in range(OB):
                fft_ib(h1r[ib], ib, b)
        OBORDER = [0, CB, 1, 1 + CB, 2, 2 + CB, 3, 3 + CB]
        ya_cur = {}
        for ob in OBORDER:
            usb = upool.tile([P, NU, OB, P], BF16, tag="uslab",
                             name=f"uslab_{ob}")
            nc.sync.dma_start(out=usb, in_=ut[ob])
            ys = conv_ob(ob, usb)
            for b in range(BLOC):
                if ob < CB:
                    ya_cur[(ob, b)] = ys[b]
                else:
                    glu(ob - CB, b, ya_cur.pop((ob - CB, b)), ys[b])
        for b in range(BLOC):
            phase_D(b)

    nc.compile()
    return nc


def prepare_inputs(x, ln_g, ln_b, w1, b1, w2, b2, bn_g, bn_b, bn_mean, bn_var, w3, b3):
    """Host-side folding + DFT weight transform + layout."""
    f = np.float32
    bf = mybir.dt.np(BF16)
    x = np.asarray(x, f)
    ln_g, ln_b = np.asarray(ln_g, f), np.asarray(ln_b, f)
    w1, b1 = np.asarray(w1, f), np.asarray(b1, f)
    w2, b2 = np.asarray(w2, f), np.asarray(b2, f)
    bn_g, bn_b = np.asarray(bn_g, f), np.asarray(bn_b, f)
    bn_mean, bn_var = np.asarray(bn_mean, f), np.asarray(bn_var, f)
    w3, b3 = np.asarray(w3, f), np.asarray(b3, f)

    # Fold LN affine into conv1, BN (eval) into conv3.
    w1f = w1 * ln_g[None, :]
    b1f = b1 + w1 @ ln_b
    s_bn = bn_g / np.sqrt(bn_var + EPS_BN)
    w3f = w3 * s_bn[None, :]
    b3f = b3 + w3 @ (bn_b - bn_mean * s_bn)

    w1d = np.ascontiguousarray(w1f.T.reshape(CB, P, 2 * C)).astype(bf)
    w3d = np.ascontiguousarray(w3f.T.reshape(CB, P, C)).astype(bf)

    # U planes: Uc = conj(FFT8(pad(w2)))/8, w2 is (K, I, O)
    wf = np.fft.fft(np.pad(w2.astype(np.float64), ((0, 8 - K), (0, 0), (0, 0))),
                    axis=0)
    Uc = np.conj(wf) / 8.0
    planes = [Uc[0].real, Uc[4].real]
    for j in (1, 2, 3):
        planes += [Uc[j].real, -Uc[j].imag, Uc[j].imag]
    ud = np.stack(planes)                      # (NU, 2C_in, 2C_out)
    ud = ud.reshape(NU, OB, P, OB, P)          # (u, ib, p, ob, o)
    ud = np.ascontiguousarray(ud.transpose(3, 2, 0, 1, 4))  # (ob, p, u, ib, o)
    ud = ud.astype(bf)

    b1d = np.ascontiguousarray(b1f.reshape(OB, P).T)
    b2d = np.ascontiguousarray(b2.reshape(OB, P).T)
    b3d = np.ascontiguousarray(np.broadcast_to(b3f, (P, C)))

    shared = {"w1t": w1d, "ut": ud, "w3t": w3d, "b1": b1d, "b2": b2d, "b3": b3d}
    in_maps = []
    for c in range(NCORES):
        m = dict(shared)
        m["xs"] = np.ascontiguousarray(x[c * BLOC:(c + 1) * BLOC]).astype(bf)
        in_maps.append(m)
    return in_maps


_NC = None
LAST_RESULTS = None


def kernel(**inputs) -> np.ndarray:
    global _NC, LAST_RESULTS
    if _NC is None:
        _NC = build_nc()
    in_maps = prepare_inputs(**inputs)
    res = run_bass_kernel_spmd(_NC, in_maps, list(range(NCORES)))
    LAST_RESULTS = res
    return np.concatenate([r["out"] for r in res.results], axis=0)


# revision 26
# speedup vs baseline: 1.3127x; 1.3127x over previous
"""Trainium2 Bass kernel for the ConvModule problem (DFT8 conv version).

Computes, for x (B=16, T=1024, C=512) fp32:
    h = LayerNorm_C(x) -> pw conv C->2C + Swish -> k=5 conv 2C->2C
      -> GLU -> BatchNorm(eval) -> pw conv C->C
Data-parallel over batch across 8 NeuronCores (2 batches/core, weights
replicated).  LN gamma/beta folded into w1/b1, BN folded into w3/b3 on the
host.

The k=5 'same' conv is computed as a length-8 cyclic correlation per tile of
4 outputs (exact since 3+4 <= 7), via a real FFT8 on the device (DVE/GpSimd
butterflies on stride-1 deinterleaved planes) and host-side transformed
weights U = conj(FFT8(w2 zero-padded))/8.  Per complex point j, three U
planes (Re, -Im, +Im) are stored so every PSUM contribution is a plain
accumulate:
    M_jr = A_j V_jr + B_j V_ji ,  M_ji = C_j V_jr + A_j V_ji
with A=Re(U), B=-Im(U), C=+Im(U).  This needs 14 GEMM passes per 4 outputs
instead of the direct method's 20, cutting Tensor-engine time ~1.4x.
"""

from contextlib import ExitStack

import numpy as np

import concourse.bass as bass
import concourse.bacc as bacc
import concourse.tile as tile
from concourse import mybir
from concourse.masks import make_identity
from concourse.bass_utils import run_bass_kernel_spmd

B, T, C, K = 16, 1024, 512, 5
EPS_LN = 1e-5
EPS_BN = 1e-5
NCORES = 8
BLOC = B // NCORES          # batches per core
P = 128                     # SBUF partitions
CB = C // P                 # 4 channel blocks of the C dim
OB = (2 * C) // P           # 8 channel blocks of the 2C dim
TH = T // 2                 # 512
NT = T // 4                 # 256 conv tiles per batch (4 outputs each)
NU = 11                     # stored U planes: U0, U4, (A,B,C) x j=1..3
F32 = mybir.dt.float32
BF16 = mybir.dt.bfloat16
RS2 = float(1.0 / np.sqrt(2.0))
SQ2 = float(np.sqrt(2.0))

AF = mybir.ActivationFunctionType
ALU = mybir.AluOpType

# GEMM pass lists: (psum plane index, [(u_idx, v_name), ...])
# u planes: 0:U0 1:U4 2:A1 3:B1 4:C1 5:A2 6:B2 7:C2 8:A3 9:B3 10:C3
# E group: M0, M4, M2r, M2i ; O group: M1r, M1i, M3r, M3i
MPASS_E = [
    (0, [(0, "v0")]),
    (1, [(1, "v4")]),
    (2, [(5, "v2r"), (6, "v2i")]),
    (3, [(7, "v2r"), (5, "v2i")]),
]
MPASS_O = [
    (0, [(2, "v1r"), (3, "v1i")]),
    (1, [(4, "v1r"), (2, "v1i")]),
    (2, [(8, "v3r"), (9, "v3i")]),
    (3, [(10, "v3r"), (8, "v3i")]),
]
VNAMES = ["v0", "v4", "v1r", "v1i", "v2r", "v2i", "v3r", "v3i"]


def build_nc() -> bass.Bass:
    nc = bacc.Bacc("TRN2")

    xs = nc.declare_dram_parameter("xs", [BLOC, T, C], BF16, isOutput=False)
    w1t = nc.declare_dram_parameter("w1t", [CB, P, 2 * C], BF16, isOutput=False)
    ut = nc.declare_dram_parameter("ut", [OB, P, NU, OB, P], BF16, isOutput=False)
    w3t = nc.declare_dram_parameter("w3t", [CB, P, C], BF16, isOutput=False)
    b1 = nc.declare_dram_parameter("b1", [P, OB], F32, isOutput=False)
    b2 = nc.declare_dram_parameter("b2", [P, OB], F32, isOutput=False)
    b3 = nc.declare_dram_parameter("b3", [P, C], F32, isOutput=False)
    out = nc.declare_dram_parameter("out", [BLOC, T, C], F32, isOutput=True)

    with ExitStack() as ctx:
        tc = ctx.enter_context(tile.TileContext(nc))

        consts = ctx.enter_context(tc.tile_pool(name="consts", bufs=1))
        xin = ctx.enter_context(tc.tile_pool(name="xin", bufs=2))
        stats = ctx.enter_context(tc.tile_pool(name="stats", bufs=4))
        hNp = ctx.enter_context(tc.tile_pool(name="hNp", bufs=1))
        h1rp = ctx.enter_context(tc.tile_pool(name="h1rp", bufs=3))
        fsc = ctx.enter_context(tc.tile_pool(name="fsc", bufs=1))
        vpool = ctx.enter_context(tc.tile_pool(name="vpool", bufs=1))
        upool = ctx.enter_context(tc.tile_pool(name="upool", bufs=2))
        mcp = ctx.enter_context(tc.tile_pool(name="mcp", bufs=2))
        isc = ctx.enter_context(tc.tile_pool(name="isc", bufs=1))
        yap = ctx.enter_context(tc.tile_pool(name="yap", bufs=2))
        hGp = ctx.enter_context(tc.tile_pool(name="hGp", bufs=1))
        outp = ctx.enter_context(tc.tile_pool(name="outp", bufs=2))
        cv_psum = ctx.enter_context(tc.tile_pool(name="cv_psum", bufs=2, space="PSUM"))
        ab_psum = ctx.enter_context(tc.tile_pool(name="ab_psum", bufs=2, space="PSUM"))
        o_psum = ctx.enter_context(tc.tile_pool(name="o_psum", bufs=2, space="PSUM"))

        # ---- constants / persistent weights ----
        ident = consts.tile([P, P], BF16, tag="ident")
        make_identity(nc, ident)
        epssb = consts.tile([P, 1], F32, tag="eps")
        nc.vector.memset(epssb, EPS_LN)
        b1sb = consts.tile([P, OB], F32, tag="b1")
        nc.sync.dma_start(out=b1sb, in_=b1[:])
        b2sb = consts.tile([P, OB], F32, tag="b2")
        nc.sync.dma_start(out=b2sb, in_=b2[:])
        b3sb = consts.tile([P, C], F32, tag="b3")
        nc.sync.dma_start(out=b3sb, in_=b3[:])
        w1sb = []
        for cb in range(CB):
            w = consts.tile([P, 2 * C], BF16, tag=f"w1_{cb}", name=f"w1_{cb}")
            nc.sync.dma_start(out=w, in_=w1t[cb])
            w1sb.append(w)
        w3sb = []
        for cb in range(CB):
            w = consts.tile([P, C], BF16, tag=f"w3_{cb}", name=f"w3_{cb}")
            nc.sync.dma_start(out=w, in_=w3t[cb])
            w3sb.append(w)

        # V planes: [128, BLOC, NT] bf16 per (plane, ib); halves written per batch
        vsb = {}
        for vn in VNAMES:
            for ib in range(OB):
                v = vpool.tile([P, BLOC, NT], BF16, tag=f"{vn}_{ib}",
                               name=f"{vn}_{ib}")
                vsb[(vn, ib)] = v

        hg_tiles = {}
        hN_all = {}

        # ---------- Phase A: LN + transpose to [c, t] ----------
        def phase_A(b):
            xh = []
            xsr = xs[b].rearrange("(h tb p) c -> h p tb c", h=4, p=P)
            for h in range(4):
                xb = xin.tile([P, T // P // 4, C], BF16, tag="xbig",
                              name=f"xbig_{b}_{h}")
                nc.gpsimd.dma_start(out=xb, in_=xsr[h])
                xh.append(xb)
            hN = hNp.tile([P, CB * T], BF16, tag="hN", name=f"hN_{b}")
            hN_all[b] = hN
            hN3 = hN[:, :].rearrange("p (c t) -> p c t", c=CB)
            for tb in range(T // P):
                xt = xh[tb // 2][:, tb % 2, :]
                st6 = stats.tile([P, 6], F32, tag="st6")
                nc.vector.bn_stats(out=st6, in_=xt)
                mv = stats.tile([P, 2], F32, tag="mv")
                nc.vector.bn_aggr(out=mv, in_=st6)
                rstd = stats.tile([P, 1], F32, tag="rstd")
                nc.scalar.activation(
                    out=rstd, in_=mv[:, 1:2], func=AF.Sqrt,
                    bias=epssb, scale=1.0,
                )
                nc.vector.reciprocal(out=rstd, in_=rstd)
                nmu = stats.tile([P, 1], F32, tag="nmu")
                nc.vector.tensor_scalar(
                    out=nmu, in0=mv[:, 0:1], scalar1=rstd, scalar2=-1.0,
                    op0=ALU.mult, op1=ALU.mult,
                )
                xn = stats.tile([P, C], BF16, tag="xn")
                nc.scalar.activation(
                    out=xn, in_=xt, func=AF.Identity, bias=nmu, scale=rstd,
                )
                ps = ab_psum.tile([P, TH], BF16, tag="ab", name=f"tp_{b}_{tb}")
                for cb in range(CB):
                    nc.tensor.transpose(
                        ps[:, cb * P:(cb + 1) * P], xn[:, cb * P:(cb + 1) * P],
                        ident,
                    )
                nc.scalar.copy(
                    out=hN3[:, :, tb * P:(tb + 1) * P],
                    in_=ps[:, 0:CB * P].rearrange("p (c i) -> p c i", c=CB),
                )

        # ---------- Phase B: conv1 C->2C + Swish into deinterleaved planes ----
        def phase_B(b):
            hN3 = hN_all[b][:, :].rearrange("p (c t) -> p c t", c=CB)
            h1r = []
            for ob in range(OB):
                # q_i[u] = h1[4u + i - 2]: 8 aligned planes (duplicated halo)
                t_ = h1rp.tile([P, 8, NT], BF16, tag="h1r",
                               name=f"h1r_{ob}_{b}")
                nc.vector.memset(t_[:, 0:2, 0:1], 0.0)
                nc.vector.memset(t_[:, 6:8, NT - 1:NT], 0.0)
                # Silu(z + b1) -> planes 2..5: t = 4u+j lands at q_{j+2}[u]
                for ph in range(2):
                    pz = ab_psum.tile([P, TH], F32, tag="ab",
                                      name=f"pz_{ob}_{b}_{ph}")
                    for cb in range(CB):
                        w = w1sb[cb][:, ob * P:(ob + 1) * P]
                        nc.tensor.matmul(
                            pz, w, hN3[:, cb, ph * TH:(ph + 1) * TH],
                            start=(cb == 0), stop=(cb == CB - 1))
                    dst = t_[:, 2:6, 128 * ph:128 * ph + 128]
                    nc.scalar.activation(
                        out=dst.rearrange("p j u -> p u j"),
                        in_=pz,
                        func=AF.Silu, bias=b1sb[:, ob:ob + 1], scale=1.0,
                    )
                # duplicated shifted planes (gpsimd copies)
                nc.vector.tensor_copy(t_[:, 0, 1:NT], t_[:, 4, 0:NT - 1])
                nc.vector.tensor_copy(t_[:, 1, 1:NT], t_[:, 5, 0:NT - 1])
                nc.vector.tensor_copy(t_[:, 6, 0:NT - 1], t_[:, 2, 1:NT])
                nc.vector.tensor_copy(t_[:, 7, 0:NT - 1], t_[:, 3, 1:NT])
                h1r.append(t_)
            return h1r

        # ---------- FFT8 per (ib, batch): h1r -> 8 V planes ----------
        def fft_ib(r, ib, b):
            d = [r[:, i, 0:NT] for i in range(8)]
            s, t_ = [], []
            for i in range(4):
                si = fsc.tile([P, NT], BF16, tag=f"s{i}", name=f"s{i}_{ib}_{b}")
                nc.vector.tensor_add(out=si, in0=d[i], in1=d[i + 4])
                s.append(si)
                ti = fsc.tile([P, NT], BF16, tag=f"t{i}", name=f"t{i}_{ib}_{b}")
                nc.vector.tensor_sub(out=ti, in0=d[i], in1=d[i + 4])
                t_.append(ti)
            u0 = fsc.tile([P, NT], BF16, tag="u0", name=f"u0_{ib}_{b}")
            nc.vector.tensor_add(out=u0, in0=s[0], in1=s[2])
            u1 = fsc.tile([P, NT], BF16, tag="u1", name=f"u1_{ib}_{b}")
            nc.vector.tensor_add(out=u1, in0=s[1], in1=s[3])
            V = {vn: vsb[(vn, ib)][:, b, :] for vn in VNAMES}
            nc.vector.tensor_add(out=V["v0"], in0=u0, in1=u1)
            nc.vector.tensor_sub(out=V["v4"], in0=u0, in1=u1)
            nc.vector.tensor_sub(out=V["v2r"], in0=s[0], in1=s[2])
            nc.vector.tensor_sub(out=V["v2i"], in0=s[3], in1=s[1])
            a = fsc.tile([P, NT], BF16, tag="fa", name=f"fa_{ib}_{b}")
            nc.vector.tensor_sub(out=a, in0=t_[1], in1=t_[3])
            bb = fsc.tile([P, NT], BF16, tag="fb", name=f"fb_{ib}_{b}")
            nc.vector.tensor_add(out=bb, in0=t_[1], in1=t_[3])
            nc.vector.scalar_tensor_tensor(
                out=V["v1r"], in0=a, scalar=RS2, in1=t_[0],
                op0=ALU.mult, op1=ALU.add)
            nc.vector.scalar_tensor_tensor(
                out=V["v3r"], in0=a, scalar=-RS2, in1=t_[0],
                op0=ALU.mult, op1=ALU.add)
            nc.vector.scalar_tensor_tensor(
                out=V["v1i"], in0=bb, scalar=-RS2, in1=t_[2],
                op0=ALU.mult, op1=ALU.subtract)
            nc.vector.scalar_tensor_tensor(
                out=V["v3i"], in0=bb, scalar=-RS2, in1=t_[2],
                op0=ALU.mult, op1=ALU.add)

        # ---------- conv GEMM + IFFT per ob (both batches share LDW) ----------
        def gemm_group(mms, passes, usb):
            for pl, plist in passes:
                n = len(plist) * OB
                i = 0
                for (ui, vn) in plist:
                    for ib in range(OB):
                        for b in range(BLOC):
                            nc.tensor.matmul(
                                mms[b][:, pl, :],
                                usb[:, ui, ib, :],
                                vsb[(vn, ib)][:, b, :],
                                start=(i == 0), stop=(i == n - 1),
                            )
                        i += 1

        def conv_ob(ob, usb):
            def stt(out_, in0, sc, in1):
                nc.vector.scalar_tensor_tensor(
                    out=out_, in0=in0, scalar=sc, in1=in1,
                    op0=ALU.mult, op1=ALU.add)

            def tl(tag, b, bufs=None):
                return isc.tile([P, NT], BF16, tag=tag, name=f"{tag}_{ob}_{b}",
                                bufs=bufs)

            mcs = [mcp.tile([P, 8, NT], BF16, tag="mc", name=f"mc_{ob}_{b}")
                   for b in range(BLOC)]
            # E group: M0, M4, M2r, M2i
            mmE = [cv_psum.tile([P, 4, NT], F32, tag="mm", name=f"mmE_{ob}_{b}")
                   for b in range(BLOC)]
            gemm_group(mmE, MPASS_E, usb)
            Es = {}
            for b in range(BLOC):
                nc.scalar.copy(out=mcs[b][:, 0:4, :], in_=mmE[b][:, :, :])
                mc = mcs[b]
                M0, M4 = mc[:, 0, :], mc[:, 1, :]
                M2r, M2i = mc[:, 2, :], mc[:, 3, :]
                Pt, Qt = tl("iP", b), tl("iQ", b)
                stt(Pt, M4, 1.0, M0)
                stt(Qt, M4, -1.0, M0)
                E = []
                for k, (msrc, sc, base) in enumerate(
                        ((M2r, 2.0, Pt), (M2i, -2.0, Qt),
                         (M2r, -2.0, Pt), (M2i, 2.0, Qt))):
                    e = tl(f"iE{k}", b, bufs=2)
                    stt(e, msrc, sc, base)
                    E.append(e)
                Es[b] = E
            # O group: M1r, M1i, M3r, M3i
            mmO = [cv_psum.tile([P, 4, NT], F32, tag="mm", name=f"mmO_{ob}_{b}")
                   for b in range(BLOC)]
            gemm_group(mmO, MPASS_O, usb)
            ys = {}
            for b in range(BLOC):
                nc.scalar.copy(out=mcs[b][:, 4:8, :], in_=mmO[b][:, :, :])
                mc = mcs[b]
                M1r, M1i = mc[:, 4, :], mc[:, 5, :]
                M3r, M3i = mc[:, 6, :], mc[:, 7, :]
                E = Es[b]
                y = yap.tile([P, 4, NT], BF16, tag="ya" if ob < CB else "yg",
                             name=f"y_{ob}_{b}")
                w0 = tl("iw0", b)
                stt(w0, M3r, 1.0, M1r)
                stt(y[:, 0, :], w0, 2.0, E[0])
                aa, bb2, e1 = tl("iaa", b), tl("ibb", b), tl("ie1", b)
                stt(aa, M1i, -1.0, M1r)          # M1r - M1i
                stt(bb2, M3i, 1.0, M3r)          # M3r + M3i
                stt(e1, bb2, -1.0, aa)           # aa - bb2
                stt(y[:, 1, :], e1, SQ2, E[1])
                w2_ = tl("iw2", b)
                stt(w2_, M1i, -1.0, M3i)         # M3i - M1i
                stt(y[:, 2, :], w2_, 2.0, E[2])
                cc, dd, e3 = tl("icc", b), tl("idd", b), tl("ie3", b)
                stt(cc, M1i, 1.0, M1r)           # M1r + M1i
                stt(dd, M3i, -1.0, M3r)          # M3r - M3i
                stt(e3, dd, -1.0, cc)            # cc - dd
                stt(y[:, 3, :], e3, -SQ2, E[3])
                ys[b] = y
            return ys

        # ---------- GLU per (value-ob v, batch) ----------
        def glu(v, b, ya, yg):
            sg = isc.tile([P, 4, NT], BF16, tag="sg", name=f"sg_{v}_{b}", bufs=2)
            nc.scalar.activation(
                out=sg.rearrange("p j u -> p (j u)"),
                in_=yg.rearrange("p j u -> p (j u)"),
                func=AF.Sigmoid, bias=b2sb[:, v + CB:v + CB + 1], scale=1.0,
            )
            hg = hGp.tile([P, T], BF16, tag=f"hg{v}", name=f"hg{v}_{b}", bufs=2)
            hg_tiles[(v, b)] = hg
            # hg[4u+j] = (ya[j,u] + b2[v]) * sg[j,u]  (scatter to t-seq layout)
            nc.vector.scalar_tensor_tensor(
                out=hg.rearrange("p (u j) -> p j u", j=4),
                in0=ya, scalar=b2sb[:, v:v + 1], in1=sg,
                op0=ALU.add, op1=ALU.mult,
            )

        # ---------- Phase D: conv3 with activations stationary ----------
        def phase_D(b):
            for tb in range(T // P):
                po = o_psum.tile([P, C], F32, tag="po", name=f"po_{b}_{tb}")
                for cb in range(CB):
                    hg = hg_tiles[(cb, b)]
                    nc.tensor.matmul(
                        po, hg[:, P * tb:P * (tb + 1)], w3sb[cb],
                        start=(cb == 0), stop=(cb == CB - 1),
                    )
                obig = outp.tile([P, C], F32, tag="obig", name=f"ob_{b}_{tb}")
                nc.vector.tensor_add(out=obig, in0=po, in1=b3sb)
                nc.gpsimd.dma_start(
                    out=out[b].rearrange("(tb p) c -> p tb c", p=P)[:, tb, :],
                    in_=obig,
                )

        # ================= schedule =================
        for b in range(BLOC):
            phase_A(b)
            h1r = phase_B(b)
            for ib in range(OB):
                fft_ib(h1r[ib], ib, b)
        OBORDER = [0, CB, 1, 1 + CB, 2, 2 + CB, 3, 3 + CB]
        ya_cur = {}
        for ob in OBORDER:
            usb = upool.tile([P, NU, OB, P], BF16, tag="uslab",
                             name=f"uslab_{ob}")
            nc.sync.dma_start(out=usb, in_=ut[ob])
            ys = conv_ob(ob, usb)
            for b in range(BLOC):
                if ob < CB:
                    ya_cur[(ob, b)] = ys[b]
                else:
                    glu(ob - CB, b, ya_cur.pop((ob - CB, b)), ys[b])
        for b in range(BLOC):
            phase_D(b)

    nc.compile()
    return nc


def prepare_inputs(x, ln_g, ln_b, w1, b1, w2, b2, bn_g, bn_b, bn_mean, bn_var, w3, b3):
    """Host-side folding + DFT weight transform + layout."""
    f = np.float32
    bf = mybir.dt.np(BF16)
    x = np.asarray(x, f)
    ln_g, ln_b = np.asarray(ln_g, f), np.asarray(ln_b, f)
    w1, b1 = np.asarray(w1, f), np.asarray(b1, f)
    w2, b2 = np.asarray(w2, f), np.asarray(b2, f)
    bn_g, bn_b = np.asarray(bn_g, f), np.asarray(bn_b, f)
    bn_mean, bn_var = np.asarray(bn_mean, f), np.asarray(bn_var, f)
    w3, b3 = np.asarray(w3, f), np.asarray(b3, f)

    # Fold LN affine into conv1, BN (eval) into conv3.
    w1f = w1 * ln_g[None, :]
    b1f = b1 + w1 @ ln_b
    s_bn = bn_g / np.sqrt(bn_var + EPS_BN)
    w3f = w3 * s_bn[None, :]
    b3f = b3 + w3 @ (bn_b - bn_mean * s_bn)

    w1d = np.ascontiguousarray(w1f.T.reshape(CB, P, 2 * C)).astype(bf)
    w3d = np.ascontiguousarray(w3f.T.reshape(CB, P, C)).astype(bf)

    # U planes: Uc = conj(FFT8(pad(w2)))/8, w2 is (K, I, O)
    wf = np.fft.fft(np.pad(w2.astype(np.float64), ((0, 8 - K), (0, 0), (0, 0))),
                    axis=0)
    Uc = np.conj(wf) / 8.0
    planes = [Uc[0].real, Uc[4].real]
    for j in (1, 2, 3):
        planes += [Uc[j].real, -Uc[j].imag, Uc[j].imag]
    ud = np.stack(planes)                      # (NU, 2C_in, 2C_out)
    ud = ud.reshape(NU, OB, P, OB, P)          # (u, ib, p, ob, o)
    ud = np.ascontiguousarray(ud.transpose(3, 2, 0, 1, 4))  # (ob, p, u, ib, o)
    ud = ud.astype(bf)

    b1d = np.ascontiguousarray(b1f.reshape(OB, P).T)
    b2d = np.ascontiguousarray(b2.reshape(OB, P).T)
    b3d = np.ascontiguousarray(np.broadcast_to(b3f, (P, C)))

    shared = {"w1t": w1d, "ut": ud, "w3t": w3d, "b1": b1d, "b2": b2d, "b3": b3d}
    in_maps = []
    for c in range(NCORES):
        m = dict(shared)
        m["xs"] = np.ascontiguousarray(x[c * BLOC:(c + 1) * BLOC]).astype(bf)
        in_maps.append(m)
    return in_maps


_NC = None
LAST_RESULTS = None


def kernel(**inputs) -> np.ndarray:
    global _NC, LAST_RESULTS
    if _NC is None:
        _NC = build_nc()
    in_maps = prepare_inputs(**inputs)
    res = run_bass_kernel_spmd(_NC, in_maps, list(range(NCORES)))
    LAST_RESULTS = res
    return np.concatenate([r["out"] for r in res.results], axis=0)


# revision 28
# speedup vs baseline: 1.5401x; 1.1733x over previous
"""Trainium2 Bass kernel for the ConvModule problem (DFT8 conv version).

Computes, for x (B=16, T=1024, C=512) fp32:
    h = LayerNorm_C(x) -> pw conv C->2C + Swish -> k=5 conv 2C->2C
      -> GLU -> BatchNorm(eval) -> pw conv C->C
Data-parallel over batch across 8 NeuronCores (2 batches/core, weights
replicated).  LN gamma/beta folded into w1/b1, BN folded into w3/b3 on the
host.

The k=5 'same' conv is a length-8 cyclic correlation per tile of 4 outputs
(exact since max tap reach 3+4 <= 7): a real FFT8 on the device (DVE
butterflies over stride-1 deinterleaved planes, plain tensor_tensor ops for
2x throughput) with host-transformed weights U = conj(FFT8(pad(w2)))/8.
The imaginary V planes are sign-flipped (Vif = -Vi) and per complex point j
three U planes {2Re, 2Im, -2Re} are stored so every PSUM contribution is a
plain accumulate:
    M_jr = 2(Re Vr + Im Vif),  M_ji = 2(Im Vr - Re Vif)
14 GEMM passes per 4 outputs instead of the direct method's 20.
"""

from contextlib import ExitStack

import numpy as np

import concourse.bass as bass
import concourse.bacc as bacc
import concourse.tile as tile
from concourse import mybir
from concourse.masks import make_identity
from concourse.bass_utils import run_bass_kernel_spmd

B, T, C, K = 16, 1024, 512, 5
EPS_LN = 1e-5
EPS_BN = 1e-5
NCORES = 8
BLOC = B // NCORES          # batches per core
P = 128                     # SBUF partitions
CB = C // P                 # 4 channel blocks of the C dim
OB = (2 * C) // P           # 8 channel blocks of the 2C dim
TH = T // 2                 # 512
NT = T // 4                 # 256 conv tiles per batch (4 outputs each)
NU = 11                     # stored U planes: U0, U4, (P1,P2,P3) x j=1..3
F32 = mybir.dt.float32
BF16 = mybir.dt.bfloat16
RS2 = float(1.0 / np.sqrt(2.0))

AF = mybir.ActivationFunctionType
ALU = mybir.AluOpType

# GEMM pass lists: (psum plane index, [(u_idx, v_name), ...])
# u planes: 0:U0 1:U4, then per j in 1..3 at 2+3(j-1): P1=2Re, P2=2Im, P3=-2Re
# E group: M0, M4, M2r, M2i ; O group: M1r, M1i, M3r, M3i
MPASS_E = [
    (0, [(0, "v0")]),
    (1, [(1, "v4")]),
    (2, [(5, "v2r"), (6, "v2if")]),
    (3, [(6, "v2r"), (7, "v2if")]),
]
MPASS_O = [
    (0, [(2, "v1r"), (3, "v1if")]),
    (1, [(3, "v1r"), (4, "v1if")]),
    (2, [(8, "v3r"), (9, "v3if")]),
    (3, [(9, "v3r"), (10, "v3if")]),
]
VNAMES = ["v0", "v4", "v1r", "v1if", "v2r", "v2if", "v3r", "v3if"]


def build_nc() -> bass.Bass:
    nc = bacc.Bacc("TRN2")

    xs = nc.declare_dram_parameter("xs", [BLOC, T, C], BF16, isOutput=False)
    w1t = nc.declare_dram_parameter("w1t", [CB, P, 2 * C], BF16, isOutput=False)
    ut = nc.declare_dram_parameter("ut", [OB, P, NU, OB, P], BF16, isOutput=False)
    w3t = nc.declare_dram_parameter("w3t", [CB, P, C], BF16, isOutput=False)
    b1 = nc.declare_dram_parameter("b1", [P, OB], F32, isOutput=False)
    b2 = nc.declare_dram_parameter("b2", [P, OB], F32, isOutput=False)
    b3 = nc.declare_dram_parameter("b3", [P, C], F32, isOutput=False)
    out = nc.declare_dram_parameter("out", [BLOC, T, C], F32, isOutput=True)

    with ExitStack() as ctx:
        tc = ctx.enter_context(tile.TileContext(nc))

        consts = ctx.enter_context(tc.tile_pool(name="consts", bufs=1))
        xin = ctx.enter_context(tc.tile_pool(name="xin", bufs=2))
        stats = ctx.enter_context(tc.tile_pool(name="stats", bufs=4))
        hNp = ctx.enter_context(tc.tile_pool(name="hNp", bufs=1))
        h1rp = ctx.enter_context(tc.tile_pool(name="h1rp", bufs=3))
        fsc = ctx.enter_context(tc.tile_pool(name="fsc", bufs=1))
        vpool = ctx.enter_context(tc.tile_pool(name="vpool", bufs=1))
        upool = ctx.enter_context(tc.tile_pool(name="upool", bufs=2))
        mcp = ctx.enter_context(tc.tile_pool(name="mcp", bufs=2))
        isc = ctx.enter_context(tc.tile_pool(name="isc", bufs=1))
        yap = ctx.enter_context(tc.tile_pool(name="yap", bufs=2))
        hGp = ctx.enter_context(tc.tile_pool(name="hGp", bufs=1))
        outp = ctx.enter_context(tc.tile_pool(name="outp", bufs=2))
        cv_psum = ctx.enter_context(tc.tile_pool(name="cv_psum", bufs=2, space="PSUM"))
        ab_psum = ctx.enter_context(tc.tile_pool(name="ab_psum", bufs=2, space="PSUM"))
        o_psum = ctx.enter_context(tc.tile_pool(name="o_psum", bufs=2, space="PSUM"))

        # ---- constants / persistent weights ----
        ident = consts.tile([P, P], BF16, tag="ident")
        make_identity(nc, ident)
        epssb = consts.tile([P, 1], F32, tag="eps")
        nc.vector.memset(epssb, EPS_LN)
        b1sb = consts.tile([P, OB], F32, tag="b1")
        nc.sync.dma_start(out=b1sb, in_=b1[:])
        b2sb = consts.tile([P, OB], F32, tag="b2")
        nc.sync.dma_start(out=b2sb, in_=b2[:])
        b3sb = consts.tile([P, C], F32, tag="b3")
        nc.sync.dma_start(out=b3sb, in_=b3[:])
        w1sb = []
        for cb in range(CB):
            w = consts.tile([P, 2 * C], BF16, tag=f"w1_{cb}", name=f"w1_{cb}")
            nc.sync.dma_start(out=w, in_=w1t[cb])
            w1sb.append(w)
        w3sb = []
        for cb in range(CB):
            w = consts.tile([P, C], BF16, tag=f"w3_{cb}", name=f"w3_{cb}")
            nc.sync.dma_start(out=w, in_=w3t[cb])
            w3sb.append(w)

        # V planes: [128, BLOC, NT] bf16 per (plane, ib); halves written per batch
        vsb = {}
        for vn in VNAMES:
            for ib in range(OB):
                v = vpool.tile([P, BLOC, NT], BF16, tag=f"{vn}_{ib}",
                               name=f"{vn}_{ib}")
                vsb[(vn, ib)] = v

        hg_tiles = {}
        hN_all = {}

        # ---------- Phase A: LN + transpose to [c, t] ----------
        def phase_A(b):
            xh = []
            xsr = xs[b].rearrange("(h tb p) c -> h p tb c", h=4, p=P)
            for h in range(4):
                xb = xin.tile([P, T // P // 4, C], BF16, tag="xbig",
                              name=f"xbig_{b}_{h}")
                nc.gpsimd.dma_start(out=xb, in_=xsr[h])
                xh.append(xb)
            hN = hNp.tile([P, CB * T], BF16, tag="hN", name=f"hN_{b}")
            hN_all[b] = hN
            hN3 = hN[:, :].rearrange("p (c t) -> p c t", c=CB)
            for tb in range(T // P):
                xt = xh[tb // 2][:, tb % 2, :]
                st6 = stats.tile([P, 6], F32, tag="st6")
                nc.vector.bn_stats(out=st6, in_=xt)
                mv = stats.tile([P, 2], F32, tag="mv")
                nc.vector.bn_aggr(out=mv, in_=st6)
                rstd = stats.tile([P, 1], F32, tag="rstd")
                nc.scalar.activation(
                    out=rstd, in_=mv[:, 1:2], func=AF.Sqrt,
                    bias=epssb, scale=1.0,
                )
                nc.vector.reciprocal(out=rstd, in_=rstd)
                nmu = stats.tile([P, 1], F32, tag="nmu")
                nc.vector.tensor_scalar(
                    out=nmu, in0=mv[:, 0:1], scalar1=rstd, scalar2=-1.0,
                    op0=ALU.mult, op1=ALU.mult,
                )
                xn = stats.tile([P, C], BF16, tag="xn")
                nc.scalar.activation(
                    out=xn, in_=xt, func=AF.Identity, bias=nmu, scale=rstd,
                )
                ps = ab_psum.tile([P, TH], BF16, tag="ab", name=f"tp_{b}_{tb}")
                for cb in range(CB):
                    nc.tensor.transpose(
                        ps[:, cb * P:(cb + 1) * P], xn[:, cb * P:(cb + 1) * P],
                        ident,
                    )
                nc.scalar.copy(
                    out=hN3[:, :, tb * P:(tb + 1) * P],
                    in_=ps[:, 0:CB * P].rearrange("p (c i) -> p c i", c=CB),
                )

        # ---------- Phase B: conv1 C->2C + Swish into deinterleaved planes ----
        def phase_B(b):
            hN3 = hN_all[b][:, :].rearrange("p (c t) -> p c t", c=CB)
            h1r = []
            for ob in range(OB):
                # q_i[u] = h1[4u + i - 2]: 8 aligned planes (duplicated halo)
                t_ = h1rp.tile([P, 8, NT], BF16, tag="h1r",
                               name=f"h1r_{ob}_{b}")
                nc.vector.memset(t_[:, 0:2, 0:1], 0.0)
                nc.vector.memset(t_[:, 6:8, NT - 1:NT], 0.0)
                # Silu(z + b1) -> planes 2..5: t = 4u+j lands at q_{j+2}[u]
                for ph in range(2):
                    pz = ab_psum.tile([P, TH], F32, tag="ab",
                                      name=f"pz_{ob}_{b}_{ph}")
                    for cb in range(CB):
                        w = w1sb[cb][:, ob * P:(ob + 1) * P]
                        nc.tensor.matmul(
                            pz, w, hN3[:, cb, ph * TH:(ph + 1) * TH],
                            start=(cb == 0), stop=(cb == CB - 1))
                    dst = t_[:, 2:6, 128 * ph:128 * ph + 128]
                    nc.scalar.activation(
                        out=dst.rearrange("p j u -> p u j"),
                        in_=pz,
                        func=AF.Silu, bias=b1sb[:, ob:ob + 1], scale=1.0,
                    )
                # duplicated shifted planes
                nc.vector.tensor_copy(t_[:, 0, 1:NT], t_[:, 4, 0:NT - 1])
                nc.vector.tensor_copy(t_[:, 1, 1:NT], t_[:, 5, 0:NT - 1])
                nc.vector.tensor_copy(t_[:, 6, 0:NT - 1], t_[:, 2, 1:NT])
                nc.vector.tensor_copy(t_[:, 7, 0:NT - 1], t_[:, 3, 1:NT])
                h1r.append(t_)
            return h1r

        # ---------- FFT8 per (ib, batch): h1r -> 8 V planes (all TT/TS) ------
        def fft_ib(r, ib, b):
            d = [r[:, i, 0:NT] for i in range(8)]
            s, t_ = [], []
            for i in range(4):
                si = fsc.tile([P, NT], BF16, tag=f"s{i}", name=f"s{i}_{ib}_{b}")
                nc.vector.tensor_add(out=si, in0=d[i], in1=d[i + 4])
                s.append(si)
                ti = fsc.tile([P, NT], BF16, tag=f"t{i}", name=f"t{i}_{ib}_{b}")
                nc.vector.tensor_sub(out=ti, in0=d[i], in1=d[i + 4])
                t_.append(ti)
            u0 = fsc.tile([P, NT], BF16, tag="u0", name=f"u0_{ib}_{b}")
            nc.vector.tensor_add(out=u0, in0=s[0], in1=s[2])
            u1 = fsc.tile([P, NT], BF16, tag="u1", name=f"u1_{ib}_{b}")
            nc.vector.tensor_add(out=u1, in0=s[1], in1=s[3])
            V = {vn: vsb[(vn, ib)][:, b, :] for vn in VNAMES}
            nc.vector.tensor_add(out=V["v0"], in0=u0, in1=u1)
            nc.vector.tensor_sub(out=V["v4"], in0=u0, in1=u1)
            nc.vector.tensor_sub(out=V["v2r"], in0=s[0], in1=s[2])
            nc.vector.tensor_sub(out=V["v2if"], in0=s[1], in1=s[3])
            a = fsc.tile([P, NT], BF16, tag="fa", name=f"fa_{ib}_{b}")
            nc.vector.tensor_sub(out=a, in0=t_[1], in1=t_[3])
            bb = fsc.tile([P, NT], BF16, tag="fb", name=f"fb_{ib}_{b}")
            nc.vector.tensor_add(out=bb, in0=t_[1], in1=t_[3])
            ap = fsc.tile([P, NT], BF16, tag="fap", name=f"fap_{ib}_{b}")
            nc.vector.tensor_scalar_mul(out=ap, in0=a, scalar1=RS2)
            bp = fsc.tile([P, NT], BF16, tag="fbp", name=f"fbp_{ib}_{b}")
            nc.vector.tensor_scalar_mul(out=bp, in0=bb, scalar1=RS2)
            nc.vector.tensor_add(out=V["v1r"], in0=t_[0], in1=ap)
            nc.vector.tensor_sub(out=V["v3r"], in0=t_[0], in1=ap)
            nc.vector.tensor_add(out=V["v1if"], in0=t_[2], in1=bp)
            nc.vector.tensor_sub(out=V["v3if"], in0=bp, in1=t_[2])

        # ---------- conv GEMM + IFFT per (ob, batch) ----------
        def gemm_group(mm, passes, b, usb):
            for pl, plist in passes:
                n = len(plist) * OB
                i = 0
                for (ui, vn) in plist:
                    for ib in range(OB):
                        nc.tensor.matmul(
                            mm[:, pl, :],
                            usb[:, ui, ib, :],
                            vsb[(vn, ib)][:, b, :],
                            start=(i == 0), stop=(i == n - 1),
                        )
                        i += 1

        def conv_ob(ob, b, usb):
            def tl(tag):
                return isc.tile([P, NT], BF16, tag=tag, name=f"{tag}_{ob}_{b}")

            mc = mcp.tile([P, 8, NT], BF16, tag="mc", name=f"mc_{ob}_{b}")
            # E group: M0, M4, M2r, M2i (x2 folded into U2 on host)
            mmE = cv_psum.tile([P, 4, NT], F32, tag="mm", name=f"mmE_{ob}_{b}")
            gemm_group(mmE, MPASS_E, b, usb)
            nc.scalar.copy(out=mc[:, 0:4, :], in_=mmE[:, :, :])
            M0, M4 = mc[:, 0, :], mc[:, 1, :]
            M2r, M2i = mc[:, 2, :], mc[:, 3, :]
            Pt, Qt = tl("iP"), tl("iQ")
            nc.vector.tensor_add(out=Pt, in0=M0, in1=M4)
            nc.vector.tensor_sub(out=Qt, in0=M0, in1=M4)
            E0, E1, E2, E3 = tl("iE0"), tl("iE1"), tl("iE2"), tl("iE3")
            nc.vector.tensor_add(out=E0, in0=Pt, in1=M2r)
            nc.vector.tensor_sub(out=E2, in0=Pt, in1=M2r)
            nc.vector.tensor_sub(out=E1, in0=Qt, in1=M2i)
            nc.vector.tensor_add(out=E3, in0=Qt, in1=M2i)
            # O group: M1r, M1i, M3r, M3i (x2 folded into U1/U3 on host)
            mmO = cv_psum.tile([P, 4, NT], F32, tag="mm", name=f"mmO_{ob}_{b}")
            gemm_group(mmO, MPASS_O, b, usb)
            nc.scalar.copy(out=mc[:, 4:8, :], in_=mmO[:, :, :])
            M1r, M1i = mc[:, 4, :], mc[:, 5, :]
            M3r, M3i = mc[:, 6, :], mc[:, 7, :]
            y = yap.tile([P, 4, NT], BF16, tag="ya" if ob < CB else "yg",
                         name=f"y_{ob}_{b}")
            w0 = tl("iw0")
            nc.vector.tensor_add(out=w0, in0=M1r, in1=M3r)
            nc.vector.tensor_add(out=y[:, 0, :], in0=E0, in1=w0)
            aa, bb2, t1v = tl("iaa"), tl("ibb"), tl("it1")
            nc.vector.tensor_sub(out=aa, in0=M1r, in1=M1i)
            nc.vector.tensor_add(out=bb2, in0=M3r, in1=M3i)
            nc.vector.tensor_sub(out=t1v, in0=aa, in1=bb2)
            nc.vector.scalar_tensor_tensor(
                out=y[:, 1, :], in0=t1v, scalar=RS2, in1=E1,
                op0=ALU.mult, op1=ALU.add)
            w2v = tl("iw2")
            nc.vector.tensor_sub(out=w2v, in0=M3i, in1=M1i)
            nc.vector.tensor_add(out=y[:, 2, :], in0=E2, in1=w2v)
            cc, dd, t3v = tl("icc"), tl("idd"), tl("it3")
            nc.vector.tensor_add(out=cc, in0=M1r, in1=M1i)
            nc.vector.tensor_sub(out=dd, in0=M3r, in1=M3i)
            nc.vector.tensor_sub(out=t3v, in0=cc, in1=dd)
            nc.vector.scalar_tensor_tensor(
                out=y[:, 3, :], in0=t3v, scalar=-RS2, in1=E3,
                op0=ALU.mult, op1=ALU.add)
            return y

        # ---------- GLU per (value-ob v, batch) ----------
        def glu(v, b, ya, yg):
            sg = isc.tile([P, 4, NT], BF16, tag="sg", name=f"sg_{v}_{b}", bufs=2)
            nc.scalar.activation(
                out=sg.rearrange("p j u -> p (j u)"),
                in_=yg.rearrange("p j u -> p (j u)"),
                func=AF.Sigmoid, bias=b2sb[:, v + CB:v + CB + 1], scale=1.0,
            )
            hg = hGp.tile([P, T], BF16, tag=f"hg{v}", name=f"hg{v}_{b}")
            hg_tiles[(v, b)] = hg
            # hg[4u+j] = (ya[j,u] + b2[v]) * sg[j,u]  (scatter to t-seq layout)
            nc.vector.scalar_tensor_tensor(
                out=hg.rearrange("p (u j) -> p j u", j=4),
                in0=ya, scalar=b2sb[:, v:v + 1], in1=sg,
                op0=ALU.add, op1=ALU.mult,
            )

        # ---------- Phase D: conv3 with activations stationary ----------
        def phase_D(b):
            for tb in range(T // P):
                po = o_psum.tile([P, C], F32, tag="po", name=f"po_{b}_{tb}")
                for cb in range(CB):
                    hg = hg_tiles[(cb, b)]
                    nc.tensor.matmul(
                        po, hg[:, P * tb:P * (tb + 1)], w3sb[cb],
                        start=(cb == 0), stop=(cb == CB - 1),
                    )
                obig = outp.tile([P, C], F32, tag="obig", name=f"ob_{b}_{tb}")
                nc.vector.tensor_add(out=obig, in0=po, in1=b3sb)
                nc.gpsimd.dma_start(
                    out=out[b].rearrange("(tb p) c -> p tb c", p=P)[:, tb, :],
                    in_=obig,
                )

        # ================= schedule =================
        for b in range(BLOC):
            phase_A(b)
            h1r = phase_B(b)
            for ib in range(OB):
                fft_ib(h1r[ib], ib, b)
        OBORDER = [0, CB, 1, 1 + CB, 2, 2 + CB, 3, 3 + CB]
        for b in range(BLOC):
            ya_cur = {}
            for ob in OBORDER:
                usb = upool.tile([P, NU, OB, P], BF16, tag="uslab",
                                 name=f"uslab_{ob}_{b}")
                nc.sync.dma_start(out=usb, in_=ut[ob])
                y = conv_ob(ob, b, usb)
                if ob < CB:
                    ya_cur[ob] = y
                else:
                    glu(ob - CB, b, ya_cur.pop(ob - CB), y)
            phase_D(b)

    nc.compile()
    return nc


def prepare_inputs(x, ln_g, ln_b, w1, b1, w2, b2, bn_g, bn_b, bn_mean, bn_var, w3, b3):
    """Host-side folding + DFT weight transform + layout."""
    f = np.float32
    bf = mybir.dt.np(BF16)
    x = np.asarray(x, f)
    ln_g, ln_b = np.asarray(ln_g, f), np.asarray(ln_b, f)
    w1, b1 = np.asarray(w1, f), np.asarray(b1, f)
    w2, b2 = np.asarray(w2, f), np.asarray(b2, f)
    bn_g, bn_b = np.asarray(bn_g, f), np.asarray(bn_b, f)
    bn_mean, bn_var = np.asarray(bn_mean, f), np.asarray(bn_var, f)
    w3, b3 = np.asarray(w3, f), np.asarray(b3, f)

    # Fold LN affine into conv1, BN (eval) into conv3.
    w1f = w1 * ln_g[None, :]
    b1f = b1 + w1 @ ln_b
    s_bn = bn_g / np.sqrt(bn_var + EPS_BN)
    w3f = w3 * s_bn[None, :]
    b3f = b3 + w3 @ (bn_b - bn_mean * s_bn)

    w1d = np.ascontiguousarray(w1f.T.reshape(CB, P, 2 * C)).astype(bf)
    w3d = np.ascontiguousarray(w3f.T.reshape(CB, P, C)).astype(bf)

    # U planes: Uc = conj(FFT8(pad(w2)))/8, w2 is (K, I, O)
    wf = np.fft.fft(np.pad(w2.astype(np.float64), ((0, 8 - K), (0, 0), (0, 0))),
                    axis=0)
    Uc = np.conj(wf) / 8.0
    planes = [Uc[0].real, Uc[4].real]
    for j in (1, 2, 3):
        planes += [2 * Uc[j].real, 2 * Uc[j].imag, -2 * Uc[j].real]
    ud = np.stack(planes)                      # (NU, 2C_in, 2C_out)
    ud = ud.reshape(NU, OB, P, OB, P)          # (u, ib, p, ob, o)
    ud = np.ascontiguousarray(ud.transpose(3, 2, 0, 1, 4))  # (ob, p, u, ib, o)
    ud = ud.astype(bf)

    b1d = np.ascontiguousarray(b1f.reshape(OB, P).T)
    b2d = np.ascontiguousarray(b2.reshape(OB, P).T)
    b3d = np.ascontiguousarray(np.broadcast_to(b3f, (P, C)))

    shared = {"w1t": w1d, "ut": ud, "w3t": w3d, "b1": b1d, "b2": b2d, "b3": b3d}
    in_maps = []
    for c in range(NCORES):
        m = dict(shared)
        m["xs"] = np.ascontiguousarray(x[c * BLOC:(c + 1) * BLOC]).astype(bf)
        in_maps.append(m)
    return in_maps


_NC = None
LAST_RESULTS = None


def kernel(**inputs) -> np.ndarray:
    global _NC, LAST_RESULTS
    if _NC is None:
        _NC = build_nc()
    in_maps = prepare_inputs(**inputs)
    res = run_bass_kernel_spmd(_NC, in_maps, list(range(NCORES)))
    LAST_RESULTS = res
    return np.concatenate([r["out"] for r in res.results], axis=0)
